# revision 7
# baseline (speedup 1.0000x reference)
"""Trainium2 Bass kernel for nn_Adj (topk_masking).

Computes, per batch b:
    si   = x_b @ x_b^T                      (512, 512)
    th_i = 32nd largest value of row i
    adj  = (si >= th)                       row degree == 32 (no boundary ties)
    out  = adj / 32                         (D^-1/2 A D^-1/2 with D = 32 I)

Sharding: pure data-parallel over batch; core i handles batches [8i, 8i+8).
The host pre-transposes x so each core receives x_b^T (C-major), which is
what the PE needs for both matmul operands (si = lhsT.T @ rhs with
lhsT = rhs = x_b^T); no on-chip transposes.
"""

import os
import sys

import numpy as np


def _import_concourse():
    try:
        import concourse.bass  # noqa: F401
        return
    except ImportError:
        pass
    for p in ("/opt/trn_rl_repo", "/root/.axon_site/_ro/trn_rl_repo"):
        if os.path.isdir(p) and p not in sys.path:
            sys.path.insert(0, p)
    import concourse.bass  # noqa: F401


B, N, C = 64, 512, 1024
K = 32
NCORES = 8
BPC = B // NCORES  # batches per core
P = 128            # SBUF partitions
KT = C // P        # contraction tiles per batch
MT = N // P        # output row tiles per batch
NEG = -1.0e30      # replacement sentinel, far below any |si| value

# matmul dtype: "f32r" runs the PE at full rate (1 cycle/row) vs plain
# fp32's 4 cycles/row; precision validated on hardware against the
# reference (see test.py).
MM_DTYPE = os.environ.get("ADJ_MM_DTYPE", "f32r")


def _build_nc(mm_dtype=MM_DTYPE):
    _import_concourse()
    import concourse.bacc as bacc
    import concourse.mybir as mybir
    from concourse.tile import TileContext

    fr = {"f32r": mybir.dt.float32r, "f32": mybir.dt.float32}[mm_dtype]

    nc = bacc.Bacc("TRN2", target_bir_lowering=False)
    # Input tensor + SBUF tiles carry the matmul dtype end-to-end (bytes are
    # identical to f32; walrus requires f32r matmul operands to be produced
    # as f32r through the whole chain).
    xt = nc.dram_tensor("xt", [BPC, C, N], fr, kind="ExternalInput")
    out = nc.dram_tensor("out", [BPC, N, N], mybir.dt.float32, kind="ExternalOutput")

    with TileContext(nc) as tc:
        with (
            tc.tile_pool(name="xtp", bufs=2) as xtp,
            tc.tile_pool(name="psp", bufs=4, space="PSUM") as psp,
            tc.tile_pool(name="sip", bufs=4) as sip,
            tc.tile_pool(name="wrkp", bufs=4) as wrkp,
            tc.tile_pool(name="v8p", bufs=8) as v8p,
            tc.tile_pool(name="mp", bufs=4) as mp,
        ):
            for b in range(BPC):
                xtb = xtp.tile([P, KT, N], fr)
                for k in range(KT):
                    nc.sync.dma_start(
                        out=xtb[:, k, :],
                        in_=xt[b, k * P:(k + 1) * P, :],
                    )
                for m in range(MT):
                    ps = psp.tile([P, N], mybir.dt.float32)
                    for k in range(KT):
                        nc.tensor.matmul(
                            ps,
                            lhsT=xtb[:, k, m * P:(m + 1) * P],
                            rhs=xtb[:, k, :],
                            start=(k == 0),
                            stop=(k == KT - 1),
                        )
                    si = sip.tile([P, N], mybir.dt.float32)
                    nc.scalar.copy(si, ps)  # PSUM -> SBUF on ACT
                    # top-32 per row: 4 rounds of max8, removing each round's
                    # 8 winners; round 4's minimum is the 32nd largest.
                    wrk = wrkp.tile([P, N], mybir.dt.float32)
                    src = si
                    v8 = None
                    for r in range(4):
                        v8 = v8p.tile([P, 8], mybir.dt.float32)
                        nc.vector.max(out=v8, in_=src)
                        if r < 3:
                            nc.vector.match_replace(
                                out=wrk, in_to_replace=v8, in_values=src,
                                imm_value=NEG,
                            )
                            src = wrk
                    msk = mp.tile([P, N], mybir.dt.float32)
                    nc.vector.tensor_scalar(
                        out=msk, in0=si, scalar1=v8[:, 7:8], scalar2=1.0 / K,
                        op0=mybir.AluOpType.is_ge, op1=mybir.AluOpType.mult,
                    )
                    nc.sync.dma_start(out=out[b, m * P:(m + 1) * P, :], in_=msk)
    nc.compile()
    return nc


_NC_CACHE = {}


def _get_nc(mm_dtype=MM_DTYPE):
    if mm_dtype not in _NC_CACHE:
        _NC_CACHE[mm_dtype] = _build_nc(mm_dtype)
    return _NC_CACHE[mm_dtype]


def _run(xt, mm_dtype=MM_DTYPE, trace=False):
    """xt: (B, C, N) float32, batch-transposed input. Returns (results, out)."""
    _import_concourse()
    from concourse.bass_utils import run_bass_kernel_spmd

    nc = _get_nc(mm_dtype)
    in_maps = [
        {"xt": np.ascontiguousarray(xt[i * BPC:(i + 1) * BPC])}
        for i in range(NCORES)
    ]
    res = run_bass_kernel_spmd(nc, in_maps, core_ids=list(range(NCORES)),
                               trace=trace)
    out = np.concatenate([res.results[i]["out"] for i in range(NCORES)], axis=0)
    return res, out


def kernel(x):
    x = np.asarray(x, dtype=np.float32)
    xt = np.ascontiguousarray(x.transpose(0, 2, 1))  # (B, C, N)
    _, out = _run(xt)
    return out
